# revision 16
# baseline (speedup 1.0000x reference)
# Negative squared Euclidean distance: out[n,c] = 2*x@p.T - ||x||^2 - ||p||^2
# x: [8192, 1024] f32, prototypes: [4096, 1024] f32 -> out: [8192, 4096] f32
#
# Strategy: data-parallel over rows of x across 8 NeuronCores. Each core gets
# 1024 rows of x and the full (replicated) prototype table. The GEMM runs in
# fp8e4m3 with DoubleRow perf mode on the TensorEngine (contraction over d on
# the partition axis, K=256 per matmul); the norms stay in exact f32 and are
# folded in during PSUM->SBUF eviction (ACT per-partition bias for ||x||^2,
# DVE broadcast-add for ||p||^2), so the dominant output term -||x||^2 is
# full precision (measured max rel err ~2.5e-4). Operands are shipped
# pre-transposed ([d, .] layout, [128, KT, *] SBUF tiles) so the device does
# no layout work. Engine split: PE matmuls / ACT eviction+bias / DVE psq-add /
# gpsimd SWDGE input loads / SP HWDGE output stores - each ring conflict-free.

from contextlib import ExitStack

import ml_dtypes
import numpy as np

N, D, C = 8192, 1024, 4096
NCORES = 8
M = N // NCORES  # rows per core

P = 128          # partition dim
NT = 512         # moving free dim / psum bank width (f32)
KT = D // P      # 8 contraction chunks
MT = M // P      # 8 row chunks per core
CT = C // NT     # 8 column panels

# fp8e4m3 GEMM with DoubleRow (2 fp8 MACs/cell/cycle). Prototypes are scaled
# by PSCALE so their ~N(0, 0.02) values sit in e4m3's normal range; the
# PSUM->SBUF activation folds 1/PSCALE back in. Norms stay exact f32.
FP8 = True
PSCALE = 32.0

_cache = {}


def _build(reps=1, fp8=FP8, ibufs=1):
    import concourse.bass as bass
    import concourse.mybir as mybir
    import concourse.tile as tile
    from concourse import bacc

    nc = bacc.Bacc(
        "TRN2",
        target_bir_lowering=False,
        debug=False,
        num_devices=NCORES,
    )

    bf16 = mybir.dt.bfloat16
    f32 = mybir.dt.float32
    mm_dt = mybir.dt.float8e4 if fp8 else bf16

    xt_d = nc.dram_tensor("xt", [D, M], mm_dt, kind="ExternalInput").ap()
    pt_d = nc.dram_tensor("pt", [D, C], mm_dt, kind="ExternalInput").ap()
    xsqn_d = nc.dram_tensor("xsqn", [P, MT], f32, kind="ExternalInput").ap()
    psqn_d = nc.dram_tensor("psqn", [P, C], f32, kind="ExternalInput").ap()
    out_d = nc.dram_tensor("out", [M, C], f32, kind="ExternalOutput").ap()

    KH = KT // 2  # half-k split for startup pipelining

    with tile.TileContext(nc) as tc, ExitStack() as ctx:
        consts = ctx.enter_context(tc.tile_pool(name="consts", bufs=1))
        psum_pool = ctx.enter_context(tc.tile_pool(name="psum", bufs=8, space="PSUM"))
        out_pool = ctx.enter_context(tc.tile_pool(name="outs", bufs=4))

        # Persistent inputs ride the (otherwise idle) gpsimd SWDGE ring as a
        # handful of big multi-dim DMAs, so neither the ACT ring (activations)
        # nor the SP ring (output stores) ever stalls on input traffic.
        # Order: xsqn, then xt halves interleaved with the first EARLY panels'
        # halves (the first tile group's operands), psqn, then the remaining
        # panels.
        xsqn_sb = consts.tile([P, MT], f32, name="xsqn_sb", tag="xsqn_sb", bufs=ibufs)
        xt3 = consts.tile([P, KT, M], mm_dt, name="xt3", tag="xt3", bufs=ibufs)
        psqn_sb = consts.tile([P, C], f32, name="psqn_sb", tag="psqn_sb", bufs=ibufs)
        pt3 = [
            consts.tile([P, KT, NT], mm_dt, name=f"pt3_{c}", tag=f"pt3_{c}", bufs=ibufs)
            for c in range(CT)
        ]

        xt_r = xt_d.rearrange("(k p) m -> p k m", p=P)
        pt_r = pt_d.rearrange("(k p) n -> p k n", p=P)
        ident = mybir.ActivationFunctionType.Identity
        out_r = out_d.rearrange("(m2 p) c -> p m2 c", p=P)  # m2 = MT pairs*2

        EARLY = 4 if fp8 else 1  # panels needed by the first tile group
        for _rep in range(reps):
            nc.gpsimd.dma_start(out=xsqn_sb[:], in_=xsqn_d[:])
            for h in range(2):
                ks = slice(h * KH, (h + 1) * KH)
                nc.gpsimd.dma_start(out=xt3[:, ks, :], in_=xt_r[:, ks, :])
                for c in range(EARLY):
                    nc.gpsimd.dma_start(
                        out=pt3[c][:, ks, :],
                        in_=pt_r[:, ks, c * NT:(c + 1) * NT],
                    )

            nc.gpsimd.dma_start(out=psqn_sb[:], in_=psqn_d[:])

            for c in range(EARLY, CT):
                nc.gpsimd.dma_start(
                    out=pt3[c][:], in_=pt_r[:, :, c * NT:(c + 1) * NT]
                )

            if fp8:
                # m outer, 4-bank c-groups, k-pair mid, c inner: 4 consecutive
                # matmuls share one stationary operand, and completions come in
                # c-order so stores batch as [128, 1024] c-pairs.
                CG = 4
                for c0 in range(0, CT, CG):
                    for m in range(MT):
                        psums = [
                            psum_pool.tile([P, NT], f32, name="psum")
                            for _ in range(CG)
                        ]
                        for k in range(0, KT, 2):
                            lhsT = xt3[:, k:k + 2, m * P:(m + 1) * P]
                            for ci in range(CG):
                                nc.tensor.matmul(
                                    psums[ci],
                                    lhsT,
                                    pt3[c0 + ci][:, k:k + 2, :],
                                    start=(k == 0),
                                    stop=(k == KT - 2),
                                    perf_mode=mybir.MatmulPerfMode.DoubleRow,
                                )
                        for ci in range(CG):
                            c = c0 + ci
                            if ci % 2 == 0:
                                sb = out_pool.tile([P, 2, NT], f32, name="sb")
                            half = sb[:, ci % 2, :]
                            nc.scalar.activation(
                                half, psums[ci], ident,
                                bias=xsqn_sb[:, m:m + 1],
                                scale=2.0 / PSCALE,
                            )
                            nc.vector.tensor_add(
                                out=half, in0=half,
                                in1=psqn_sb[:, c * NT:(c + 1) * NT],
                            )
                            if ci % 2 == 1:
                                nc.sync.dma_start(
                                    out=out_d[m * P:(m + 1) * P,
                                              (c - 1) * NT:(c + 1) * NT],
                                    in_=sb[:].rearrange("p two n -> p (two n)"),
                                )
            else:
                for c in range(CT):
                    cs = slice(c * NT, (c + 1) * NT)
                    for m in range(MT):
                        psum = psum_pool.tile([P, NT], f32, name="psum")
                        for k in range(KT):
                            nc.tensor.matmul(
                                psum[:],
                                xt3[:, k, m * P:(m + 1) * P],
                                pt3[c][:, k, :],
                                start=(k == 0),
                                stop=(k == KT - 1),
                            )
                        if m % 2 == 0:
                            sb = out_pool.tile([P, 2, NT], f32, name="sb")
                        half = sb[:, m % 2, :]
                        # half = 2*psum - x_sq[m-chunk]  (per-partition f32 bias)
                        nc.scalar.activation(
                            half, psum[:], ident,
                            bias=xsqn_sb[:, m:m + 1], scale=2.0,
                        )
                        # half -= p_sq[c-slice] (replicated across partitions)
                        nc.vector.tensor_add(out=half, in0=half, in1=psqn_sb[:, cs])
                        if m % 2 == 1:
                            # store the (m-1, m) pair as one DMA
                            nc.sync.dma_start(
                                out=out_r[:, m - 1:m + 1, cs], in_=sb[:]
                            )

    nc.compile()
    return nc


def _prep_inputs(x, prototypes, fp8=FP8):
    mm_np = ml_dtypes.float8_e4m3 if fp8 else ml_dtypes.bfloat16
    x = np.asarray(x, dtype=np.float32)
    prototypes = np.asarray(prototypes, dtype=np.float32)

    p_mm = prototypes * np.float32(PSCALE) if fp8 else prototypes
    pt = np.ascontiguousarray(p_mm.T).astype(mm_np)                    # [D, C]
    psq = (prototypes.astype(np.float64) ** 2).sum(axis=1)
    psqn = np.ascontiguousarray(
        np.broadcast_to(-psq[None, :].astype(np.float32), (P, C))
    )

    in_maps = []
    for i in range(NCORES):
        s = x[i * M:(i + 1) * M]                                       # [M, D]
        xt = np.ascontiguousarray(s.T).astype(mm_np)                   # [D, M]
        xsq = (s.astype(np.float64) ** 2).sum(axis=1).astype(np.float32)
        xsqn = np.ascontiguousarray(-xsq.reshape(MT, P).T)             # [P, MT]
        in_maps.append({"xt": xt, "pt": pt, "xsqn": xsqn, "psqn": psqn})
    return in_maps


def run(inputs, trace=False, tmpdir=None):
    """Build (cached), run on 8 cores, return (out, BassKernelResults)."""
    from concourse.bass_utils import run_bass_kernel_spmd

    if "nc" not in _cache:
        _cache["nc"] = _build()
    nc = _cache["nc"]

    in_maps = _prep_inputs(inputs["x"], inputs["prototypes"])
    res = run_bass_kernel_spmd(
        nc, in_maps, core_ids=list(range(NCORES)), trace=trace, tmpdir=tmpdir
    )
    out = np.concatenate([res.results[i]["out"] for i in range(NCORES)], axis=0)
    return out, res


def kernel(**inputs):
    out, _ = run(inputs, trace=False)
    return out


# revision 19
# speedup vs baseline: 1.0773x; 1.0773x over previous
# Negative squared Euclidean distance: out[n,c] = 2*x@p.T - ||x||^2 - ||p||^2
# x: [8192, 1024] f32, prototypes: [4096, 1024] f32 -> out: [8192, 4096] f32
#
# Strategy: data-parallel over rows of x across 8 NeuronCores. Each core gets
# 1024 rows of x and the full (replicated) prototype table. The GEMM runs in
# fp8e4m3 with DoubleRow perf mode on the TensorEngine (contraction over d on
# the partition axis, K=256 per matmul); the norms stay in exact f32 and are
# folded in during PSUM->SBUF eviction (ACT per-partition bias for ||x||^2,
# DVE broadcast-add for ||p||^2), so the dominant output term -||x||^2 is
# full precision (measured max rel err ~2.5e-4). Operands are shipped
# pre-transposed ([d, .] layout, [128, KT, *] SBUF tiles) so the device does
# no layout work. Engine split: PE matmuls / ACT eviction+bias / DVE psq-add /
# gpsimd SWDGE input loads / SP HWDGE output stores - each ring conflict-free.

from contextlib import ExitStack

import ml_dtypes
import numpy as np

N, D, C = 8192, 1024, 4096
NCORES = 8
M = N // NCORES  # rows per core

P = 128          # partition dim
NT = 512         # moving free dim / psum bank width (f32)
KT = D // P      # 8 contraction chunks
MT = M // P      # 8 row chunks per core
CT = C // NT     # 8 column panels

# fp8e4m3 GEMM with DoubleRow (2 fp8 MACs/cell/cycle). Prototypes are scaled
# by PSCALE so their ~N(0, 0.02) values sit in e4m3's normal range; the
# PSUM->SBUF activation folds 1/PSCALE back in. Norms stay exact f32.
FP8 = True
PSCALE = 32.0

_cache = {}


def _build(reps=1, fp8=FP8, ibufs=1):
    import concourse.bass as bass
    import concourse.mybir as mybir
    import concourse.tile as tile
    from concourse import bacc

    nc = bacc.Bacc(
        "TRN2",
        target_bir_lowering=False,
        debug=False,
        num_devices=NCORES,
    )

    bf16 = mybir.dt.bfloat16
    f32 = mybir.dt.float32
    mm_dt = mybir.dt.float8e4 if fp8 else bf16

    xt_d = nc.dram_tensor("xt", [D, M], mm_dt, kind="ExternalInput").ap()
    pt_d = nc.dram_tensor("pt", [D, C], mm_dt, kind="ExternalInput").ap()
    xsqn_d = nc.dram_tensor("xsqn", [P, MT], f32, kind="ExternalInput").ap()
    psqn_d = nc.dram_tensor("psqn", [P, C], f32, kind="ExternalInput").ap()
    out_d = nc.dram_tensor("out", [M, C], f32, kind="ExternalOutput").ap()

    KH = KT // 2  # half-k split for startup pipelining

    with tile.TileContext(nc) as tc, ExitStack() as ctx:
        consts = ctx.enter_context(tc.tile_pool(name="consts", bufs=1))
        psum_pool = ctx.enter_context(tc.tile_pool(name="psum", bufs=8, space="PSUM"))
        out_pool = ctx.enter_context(tc.tile_pool(name="outs", bufs=4))

        # Persistent inputs ride the (otherwise idle) gpsimd SWDGE ring as a
        # handful of big multi-dim DMAs, so neither the ACT ring (activations)
        # nor the SP ring (output stores) ever stalls on input traffic.
        # Order: xsqn, then xt halves interleaved with the first EARLY panels'
        # halves (the first tile group's operands), psqn, then the remaining
        # panels.
        xsqn_sb = consts.tile([P, MT], f32, name="xsqn_sb", tag="xsqn_sb", bufs=ibufs)
        xt3 = consts.tile([P, KT, M], mm_dt, name="xt3", tag="xt3", bufs=ibufs)
        psqn_sb = consts.tile([P, C], f32, name="psqn_sb", tag="psqn_sb", bufs=ibufs)
        pt3 = [
            consts.tile([P, KT, NT], mm_dt, name=f"pt3_{c}", tag=f"pt3_{c}", bufs=ibufs)
            for c in range(CT)
        ]

        xt_r = xt_d.rearrange("(k p) m -> p k m", p=P)
        pt_r = pt_d.rearrange("(k p) n -> p k n", p=P)
        ident = mybir.ActivationFunctionType.Identity
        out_r = out_d.rearrange("(m2 p) c -> p m2 c", p=P)  # m2 = MT pairs*2

        CG = 2 if fp8 else 1     # psum banks per tile group
        EARLY = CG if fp8 else 1  # panels needed by the first tile group
        for _rep in range(reps):
            # Low-latency HWDGE (sync ring) prefetch of the first k-pair
            # slices + xsqn; bulk rides the gpsimd SWDGE ring on disjoint
            # slices so nothing serializes.
            nc.sync.dma_start(out=xt3[:, 0:2, :], in_=xt_r[:, 0:2, :])
            for c in range(EARLY):
                nc.sync.dma_start(
                    out=pt3[c][:, 0:2, :],
                    in_=pt_r[:, 0:2, c * NT:(c + 1) * NT],
                )
            nc.sync.dma_start(out=xsqn_sb[:], in_=xsqn_d[:])

            nc.gpsimd.dma_start(out=xt3[:, 2:KT, :], in_=xt_r[:, 2:KT, :])
            for c in range(EARLY):
                nc.gpsimd.dma_start(
                    out=pt3[c][:, 2:KT, :],
                    in_=pt_r[:, 2:KT, c * NT:(c + 1) * NT],
                )
            nc.gpsimd.dma_start(out=psqn_sb[:], in_=psqn_d[:])

            for c in range(EARLY, CT):
                nc.gpsimd.dma_start(
                    out=pt3[c][:], in_=pt_r[:, :, c * NT:(c + 1) * NT]
                )

            if fp8:
                # m outer, CG-bank c-groups, k-pair mid, c inner; completions
                # come in c-order so stores batch as [128, 1024] c-pairs.
                for c0 in range(0, CT, CG):
                    for m in range(MT):
                        psums = [
                            psum_pool.tile([P, NT], f32, name="psum")
                            for _ in range(CG)
                        ]
                        for k in range(0, KT, 2):
                            lhsT = xt3[:, k:k + 2, m * P:(m + 1) * P]
                            for ci in range(CG):
                                nc.tensor.matmul(
                                    psums[ci],
                                    lhsT,
                                    pt3[c0 + ci][:, k:k + 2, :],
                                    start=(k == 0),
                                    stop=(k == KT - 2),
                                    perf_mode=mybir.MatmulPerfMode.DoubleRow,
                                )
                        for ci in range(CG):
                            c = c0 + ci
                            if ci % 2 == 0:
                                sb = out_pool.tile([P, 2, NT], f32, name="sb")
                            half = sb[:, ci % 2, :]
                            nc.scalar.activation(
                                half, psums[ci], ident,
                                bias=xsqn_sb[:, m:m + 1],
                                scale=2.0 / PSCALE,
                            )
                            nc.vector.tensor_add(
                                out=half, in0=half,
                                in1=psqn_sb[:, c * NT:(c + 1) * NT],
                            )
                            if ci % 2 == 1:
                                nc.sync.dma_start(
                                    out=out_d[m * P:(m + 1) * P,
                                              (c - 1) * NT:(c + 1) * NT],
                                    in_=sb[:].rearrange("p two n -> p (two n)"),
                                )
            else:
                for c in range(CT):
                    cs = slice(c * NT, (c + 1) * NT)
                    for m in range(MT):
                        psum = psum_pool.tile([P, NT], f32, name="psum")
                        for k in range(KT):
                            nc.tensor.matmul(
                                psum[:],
                                xt3[:, k, m * P:(m + 1) * P],
                                pt3[c][:, k, :],
                                start=(k == 0),
                                stop=(k == KT - 1),
                            )
                        if m % 2 == 0:
                            sb = out_pool.tile([P, 2, NT], f32, name="sb")
                        half = sb[:, m % 2, :]
                        # half = 2*psum - x_sq[m-chunk]  (per-partition f32 bias)
                        nc.scalar.activation(
                            half, psum[:], ident,
                            bias=xsqn_sb[:, m:m + 1], scale=2.0,
                        )
                        # half -= p_sq[c-slice] (replicated across partitions)
                        nc.vector.tensor_add(out=half, in0=half, in1=psqn_sb[:, cs])
                        if m % 2 == 1:
                            # store the (m-1, m) pair as one DMA
                            nc.sync.dma_start(
                                out=out_r[:, m - 1:m + 1, cs], in_=sb[:]
                            )

    nc.compile()
    return nc


def _prep_inputs(x, prototypes, fp8=FP8):
    mm_np = ml_dtypes.float8_e4m3 if fp8 else ml_dtypes.bfloat16
    x = np.asarray(x, dtype=np.float32)
    prototypes = np.asarray(prototypes, dtype=np.float32)

    p_mm = prototypes * np.float32(PSCALE) if fp8 else prototypes
    pt = np.ascontiguousarray(p_mm.T).astype(mm_np)                    # [D, C]
    psq = (prototypes.astype(np.float64) ** 2).sum(axis=1)
    psqn = np.ascontiguousarray(
        np.broadcast_to(-psq[None, :].astype(np.float32), (P, C))
    )

    in_maps = []
    for i in range(NCORES):
        s = x[i * M:(i + 1) * M]                                       # [M, D]
        xt = np.ascontiguousarray(s.T).astype(mm_np)                   # [D, M]
        xsq = (s.astype(np.float64) ** 2).sum(axis=1).astype(np.float32)
        xsqn = np.ascontiguousarray(-xsq.reshape(MT, P).T)             # [P, MT]
        in_maps.append({"xt": xt, "pt": pt, "xsqn": xsqn, "psqn": psqn})
    return in_maps


def run(inputs, trace=False, tmpdir=None):
    """Build (cached), run on 8 cores, return (out, BassKernelResults)."""
    from concourse.bass_utils import run_bass_kernel_spmd

    if "nc" not in _cache:
        _cache["nc"] = _build()
    nc = _cache["nc"]

    in_maps = _prep_inputs(inputs["x"], inputs["prototypes"])
    res = run_bass_kernel_spmd(
        nc, in_maps, core_ids=list(range(NCORES)), trace=trace, tmpdir=tmpdir
    )
    out = np.concatenate([res.results[i]["out"] for i in range(NCORES)], axis=0)
    return out, res


def kernel(**inputs):
    out, _ = run(inputs, trace=False)
    return out
